# revision 1
# baseline (speedup 1.0000x reference)
"""Trainium2 Bass kernel for DGNN message passing (scatter-softmax GNN).

Math (reference):
    src, dst = edge_index[0], edge_index[2]
    alpha_e  = <entities[src_e], entities[dst_e]> / sqrt(256)
    attn     = scatter_softmax(alpha, dst)
    out[n]   = sum_{e: dst_e = n} attn_e * entities[src_e]

Sharding: destination nodes range-partitioned over 8 cores (12500 each);
edges bucketed by destination node tile (128 nodes) so each core computes
its output slice independently (no collectives).

Per-core pipeline (all engines overlap; GPSIMD descriptor generation for
the source-row gather is the critical path):
  - qv rows gathered with dma_gather (4 SWDGE queues). int16 indices force
    a 4-way bank split of the entities table; edge slots are grouped by
    (node tile, src bank) with cross-core-uniform capacities.
  - scores A[e,n] = qv . entities[node] need no k-gather: per 128-edge
    tile, lhsT = qvT (PE transpose of gathered qv), rhs = a 128-column
    slice of the CPU-pretransposed local node table (resident in SBUF).
  - M[e,n] = (local_dst[e]==n) * exp(A[e,n]*scale): indicator built with
    one broadcast-AP tensor_tensor is_equal, exp on the scalar engine
    (|alpha| < 5 for this data so no max subtraction is needed), masked
    multiply on the vector engine.
  - One PSUM tile per node tile accumulates [weighted sum | segment sum]
    via two matmuls sharing lhsT = M (rhs = qv, rhs = ones column).
  - out = W / (segsum + eps), eps preserves zeros for isolated nodes.
"""

import math

import numpy as np

import concourse.bacc as bacc
import concourse.bass as bass
import concourse.mybir as mybir
from concourse.tile import TileContext
from concourse.masks import make_identity
from concourse.bass_utils import run_bass_kernel_spmd

P = 128
D = 128
HIDDEN_DIM = 128
SCALE = 1.0 / math.sqrt(D + HIDDEN_DIM)

N_CORES = 8
N_FULL = 100000
NPC = N_FULL // N_CORES  # 12500 destination nodes per core
NT = (NPC + P - 1) // P  # 98 node tiles per core
NLOC = NT * P  # 12544 padded local nodes
N_BANKS = 4
BANK = 25000  # bank rows (< 32768 so int16 indices work)
EPS = 1e-20
WIN = 2  # node tiles per gather window


def _prep_shards(src, dst):
    """Bucket edges by (core, node tile, src bank); build slot arrays.

    Slot space per core: node tiles in order; within a node tile, N_BANKS
    groups each padded to a multiple of 128 slots with cross-core-uniform
    chunk counts nch[t][b] (so one NEFF fits all cores). Slot i of a group
    -> partition i%128, chunk i//128 (dma_gather's output order).

    Returns (nch, shards): nch [NT, N_BANKS] int; shards per core with
      qidx:  [128, total_chunks*8] int16 gather indices (bank-local,
             wrapped 16 partitions, replicated to 8 gpsimd cores)
      dstl:  [128, total_chunks] float32 local dst id per slot (-1 = pad)
    """
    core = dst // NPC
    t_in_core = (dst - core * NPC) >> 7
    b_of_edge = src // BANK
    # order edges by (core, tile, bank), stable
    key = (core * NT + t_in_core) * N_BANKS + b_of_edge
    order = np.argsort(key, kind="stable")
    key_s = key[order]
    counts = np.bincount(key, minlength=N_CORES * NT * N_BANKS).reshape(
        N_CORES, NT, N_BANKS
    )
    nch = np.ceil(counts.max(axis=0) / P).astype(np.int64)  # [NT, N_BANKS]
    nch = np.maximum(nch, 1)
    group_chunk_off = np.concatenate([[0], np.cumsum(nch.ravel())])  # flat (t,b)
    total_chunks = int(group_chunk_off[-1])

    starts = np.zeros(N_CORES * NT * N_BANKS, dtype=np.int64)
    np.cumsum(
        np.bincount(key, minlength=N_CORES * NT * N_BANKS)[:-1], out=starts[1:]
    )
    offs = np.arange(len(order), dtype=np.int64) - starts[key_s]

    src_s = src[order].astype(np.int64)
    dst_s = dst[order].astype(np.int64)
    core_s = core[order]
    tb_flat = (t_in_core[order] * N_BANKS + b_of_edge[order]).astype(np.int64)
    slot = group_chunk_off[tb_flat] * P + offs  # global slot id within core
    loc = (dst_s - core_s * NPC) & 127  # local id within node tile

    shards = []
    for c in range(N_CORES):
        m = core_s == c
        qidx = np.zeros((16, total_chunks * 8), np.int16)
        dstl = np.full((P, total_chunks), -1.0, np.float32)
        s = slot[m]
        # gather index wrap: within each (t,b) group, index i (group-local)
        # lives at partition i%16, column gbase*8 + i//16
        gl = offs[m]  # group-local position
        gcol = group_chunk_off[tb_flat[m]] * 8 + gl // 16
        qidx[gl % 16, gcol] = (src_s[m] - b_of_edge[order][m] * BANK).astype(
            np.int16
        )
        dstl[s % P, s // P] = loc[m]
        shards.append(
            {
                "qidx": np.tile(qidx, (8, 1)),
                "dstl": dstl,
            }
        )
    return nch, shards


def build_program(nch):
    """Build the SPMD Bass program. nch: [NT, N_BANKS] chunk counts."""
    total_chunks = int(nch.sum())
    nc = bacc.Bacc(None, target_bir_lowering=False, num_swdge_queues=4)
    entities = nc.dram_tensor(
        "entities", [N_FULL, D], mybir.dt.float32, kind="ExternalInput"
    )
    ntT = nc.dram_tensor("ntT", [P, NLOC], mybir.dt.float32, kind="ExternalInput")
    qidx = nc.dram_tensor(
        "qidx", [P, total_chunks * 8], mybir.dt.int16, kind="ExternalInput"
    )
    dstl = nc.dram_tensor(
        "dstl", [P, total_chunks], mybir.dt.float32, kind="ExternalInput"
    )
    out = nc.dram_tensor("out", [NLOC, D], mybir.dt.float32, kind="ExternalOutput")

    # per-(t,b) chunk offsets into the slot space
    goff = np.concatenate([[0], np.cumsum(nch.ravel())]).astype(int)
    tile_chunks = nch.sum(axis=1).astype(int)  # chunks per node tile
    t_chunk_off = np.concatenate([[0], np.cumsum(tile_chunks)]).astype(int)

    qn = 0
    with TileContext(nc) as tc:
        with (
            tc.tile_pool(name="const_pool", bufs=1) as cpool,
            tc.tile_pool(name="idx_pool", bufs=1) as ipool,
            tc.tile_pool(name="gather_pool", bufs=2) as gpool,
            tc.tile_pool(name="qvt_pool", bufs=2) as qpool,
            tc.tile_pool(name="ind_pool", bufs=2) as indpool,
            tc.tile_pool(name="m_pool", bufs=3) as mpool,
            tc.tile_pool(name="work_pool", bufs=4) as wpool,
            tc.tile_pool(name="out_pool", bufs=3) as opool,
            tc.tile_pool(name="pt_pool", bufs=2, space="PSUM") as ptpool,
            tc.tile_pool(name="pa_pool", bufs=2, space="PSUM") as papool,
            tc.tile_pool(name="pw_pool", bufs=2, space="PSUM") as pwpool,
            tc.tile_pool(name="ps_pool", bufs=2, space="PSUM") as pspool,
        ):
            identity = cpool.tile([P, P], mybir.dt.float32)
            make_identity(nc, identity[:])
            iota_i = cpool.tile([P, P], mybir.dt.int32)
            nc.gpsimd.iota(iota_i[:], pattern=[[1, P]], base=0, channel_multiplier=0)
            iota_f = cpool.tile([P, P], mybir.dt.float32)
            nc.vector.tensor_copy(iota_f[:], iota_i[:])
            ones = cpool.tile([P, 1], mybir.dt.float32)
            nc.vector.memset(ones[:], 1.0)

            ntT_sb = ipool.tile([P, NLOC], mybir.dt.float32)
            nc.sync.dma_start(out=ntT_sb[:], in_=ntT[:])
            dstl_sb = ipool.tile([P, total_chunks], mybir.dt.float32)
            nc.sync.dma_start(out=dstl_sb[:], in_=dstl[:])
            qidx_sb = ipool.tile([P, total_chunks * 8], mybir.dt.int16)
            nc.sync.dma_start(out=qidx_sb[:], in_=qidx[:])

            for t0 in range(0, NT, WIN):
                nts = list(range(t0, min(t0 + WIN, NT)))
                wch = int(sum(tile_chunks[t] for t in nts))  # window chunks
                c0 = int(t_chunk_off[t0])  # first chunk of window

                qv = gpool.tile([P, wch, D], mybir.dt.float32, tag="qv", name="qv")
                for t in nts:
                    for b in range(N_BANKS):
                        g = t * N_BANKS + b
                        gc0 = int(goff[g]) - c0  # window-local chunk offset
                        gn = int(nch[t, b])
                        ni = gn * P
                        nc.gpsimd.dma_gather(
                            qv[:, gc0 : gc0 + gn, :],
                            entities[b * BANK : min((b + 1) * BANK, N_FULL), :],
                            qidx_sb[:, (int(goff[g])) * 8 : (int(goff[g]) + gn) * 8],
                            ni,
                            ni,
                            D,
                            single_packet=False,
                            queue_num=qn % 4,
                        )
                        qn += 1

                # indicator for the whole window: ind[p, c, n] = (dstl[p,c]==n)
                ind = indpool.tile([P, wch, P], mybir.dt.float32, tag="ind", name="ind")
                nc.vector.tensor_tensor(
                    out=ind[:],
                    in0=dstl_sb[:, c0 : c0 + wch, None].to_broadcast([P, wch, P]),
                    in1=iota_f[:, None, :].to_broadcast([P, wch, P]),
                    op=mybir.AluOpType.is_equal,
                )

                # transpose qv tiles (batches of 4 into one PSUM bank)
                qvT = qpool.tile([P, wch * P], mybir.dt.float32, tag="qvT", name="qvT")
                for g0 in range(0, wch, 4):
                    gsz = min(4, wch - g0)
                    tp = ptpool.tile([P, 512], mybir.dt.float32, tag="tp", name="tp")
                    for j in range(gsz):
                        nc.tensor.transpose(
                            tp[:, j * P : (j + 1) * P],
                            qv[:, g0 + j, :],
                            identity[:],
                        )
                    nc.scalar.copy(
                        qvT[:, g0 * P : (g0 + gsz) * P], tp[:, : gsz * P]
                    )

                # per node tile: scores, masked exp, accumulate
                for t in nts:
                    tc0 = int(t_chunk_off[t]) - c0  # window-local first chunk
                    tnch = int(tile_chunks[t])
                    wps = pwpool.tile([P, D], mybir.dt.float32, tag="wps", name="wps")
                    seg = pspool.tile([P, 1], mybir.dt.float32, tag="seg", name="seg")
                    for g0 in range(0, tnch, 4):
                        gsz = min(4, tnch - g0)
                        ap = papool.tile(
                            [P, 512], mybir.dt.float32, tag="ap", name="ap"
                        )
                        for j in range(gsz):
                            cj = tc0 + g0 + j
                            nc.tensor.matmul(
                                ap[:, j * P : (j + 1) * P],
                                lhsT=qvT[:, cj * P : (cj + 1) * P],
                                rhs=ntT_sb[:, t * P : (t + 1) * P],
                                start=True,
                                stop=True,
                            )
                        expa = wpool.tile(
                            [P, 512], mybir.dt.float32, tag="expa", name="expa"
                        )
                        nc.scalar.activation(
                            expa[:, : gsz * P],
                            ap[:, : gsz * P],
                            mybir.ActivationFunctionType.Exp,
                            scale=SCALE,
                        )
                        msel = mpool.tile(
                            [P, 512], mybir.dt.float32, tag="msel", name="msel"
                        )
                        nc.vector.tensor_tensor(
                            out=msel[:, : gsz * P],
                            in0=expa[:, : gsz * P],
                            in1=ind[:, tc0 + g0 : tc0 + g0 + gsz, :],
                            op=mybir.AluOpType.mult,
                        )
                        for j in range(gsz):
                            cj = tc0 + g0 + j
                            first = g0 + j == 0
                            last = g0 + j == tnch - 1
                            nc.tensor.matmul(
                                wps[:],
                                lhsT=msel[:, j * P : (j + 1) * P],
                                rhs=qv[:, cj, :],
                                start=first,
                                stop=last,
                            )
                            nc.tensor.matmul(
                                seg[:],
                                lhsT=msel[:, j * P : (j + 1) * P],
                                rhs=ones[:],
                                start=first,
                                stop=last,
                            )
                    denom = wpool.tile([P, 1], mybir.dt.float32, tag="den", name="den")
                    nc.vector.tensor_scalar_add(denom[:], seg[:], EPS)
                    recip = wpool.tile([P, 1], mybir.dt.float32, tag="rec", name="rec")
                    nc.vector.reciprocal(recip[:], denom[:])
                    ot = opool.tile([P, D], mybir.dt.float32, tag="ot", name="ot")
                    nc.scalar.activation(
                        ot[:],
                        wps[:],
                        mybir.ActivationFunctionType.Copy,
                        scale=recip[:],
                    )
                    nc.sync.dma_start(out=out[t * P : (t + 1) * P, :], in_=ot[:])
    nc.compile()
    return nc


def kernel(entities, relations, edge_index, _trace=False):
    entities = np.ascontiguousarray(entities, dtype=np.float32)
    src = np.asarray(edge_index[0], dtype=np.int64)
    dst = np.asarray(edge_index[2], dtype=np.int64)
    assert entities.shape == (N_FULL, D)

    nch, shards = _prep_shards(src, dst)
    nc = build_program(nch)

    in_maps = []
    for c in range(N_CORES):
        ntT_c = np.ascontiguousarray(
            np.pad(
                entities[c * NPC : (c + 1) * NPC], ((0, NLOC - NPC), (0, 0))
            ).T
        )
        in_maps.append(
            {
                "entities": entities,
                "ntT": ntT_c,
                "qidx": shards[c]["qidx"],
                "dstl": shards[c]["dstl"],
            }
        )
    res = run_bass_kernel_spmd(
        nc, in_maps, core_ids=list(range(N_CORES)), trace=_trace
    )
    out = np.concatenate([r["out"][:NPC] for r in res.results], axis=0)
    if _trace:
        kernel.last_results = res
    return out



# revision 7
# speedup vs baseline: 1.8937x; 1.8937x over previous
"""Trainium2 Bass kernel for DGNN message passing (scatter-softmax GNN).

Math (reference):
    src, dst = edge_index[0], edge_index[2]
    alpha_e  = <entities[src_e], entities[dst_e]> / sqrt(256)
    attn     = scatter_softmax(alpha, dst)
    out[n]   = sum_{e: dst_e = n} attn_e * entities[src_e]

Sharding: destination nodes range-partitioned over 8 cores (12500 each);
edges bucketed by destination node tile (128 nodes) so each core computes
its output slice independently (no collectives).

v2 design (vs v1 baseline at 1.26 ms):
  - bf16 everywhere on the hot path: the entities table is pre-cast to
    bf16 so the per-edge row gather moves 256B rows (half the HBM/DMA
    traffic) and every PE matmul runs at 1 cycle/row instead of fp32's 4.
  - Gathers merged per (window, bank): slot space is ordered
    window-major, bank-major inside a window, tile-major inside a bank
    run. One dma_gather covers a whole (window, bank) run, cutting SWDGE
    descriptor-generation instructions from 392 to ~52 (994 ns fixed cost
    each, serialized on the GPSIMD engine).
  - Per 128-edge chunk on PE: transpose (qv -> qvT), scores matmul
    (qvT^T @ ntT tile), then weighted-sum + segment-sum matmuls sharing
    lhsT = msel into one PSUM accumulator [out | seg].
  - Indicator is_equal built once per window on DVE; exp on ACT; masked
    multiply on DVE; final out = accW * (1/(seg+eps)) on ACT.
"""

import math

import ml_dtypes
import numpy as np

import concourse.bacc as bacc
import concourse.mybir as mybir
from concourse.tile import TileContext
from concourse.masks import make_identity
from concourse.bass_utils import run_bass_kernel_spmd

P = 128
D = 128
HIDDEN_DIM = 128
SCALE = 1.0 / math.sqrt(D + HIDDEN_DIM)

N_CORES = 8
N_FULL = 100000
NPC = N_FULL // N_CORES  # 12500 destination nodes per core
NT = (NPC + P - 1) // P  # 98 node tiles per core
NLOC = NT * P  # 12544 padded local nodes
N_BANKS = 4
BANK = 25000  # bank rows (< 32768 so int16 indices work)
EPS = 1e-20
WIN = 8  # node tiles per gather window
NW = (NT + WIN - 1) // WIN  # windows per core
BF16 = ml_dtypes.bfloat16


def _prep_shards(src, dst):
    """Bucket edges by (core, window, bank, tile); build slot arrays.

    Slot space per core: windows in order; within a window, N_BANKS bank
    runs; within a bank run, the window's tiles in order, each (t, b)
    group padded to a multiple of 128 slots with cross-core-uniform chunk
    counts nch[t][b] (so one NEFF fits all cores). Slot s -> partition
    s%128, chunk s//128 (dma_gather's output order; gather boundaries are
    128-multiples so the global formula holds).

    Returns (nch, shards): nch [NT, N_BANKS] int; shards per core with
      qidx:  [128, total_chunks*8] int16 gather indices (bank-local,
             wrapped 16 partitions, replicated to 8 gpsimd cores)
      dstl:  [128, total_chunks] bf16 tile-local dst per slot (-1 = pad)
    """
    core = dst // NPC
    t_in_core = (dst - core * NPC) >> 7
    b_of_edge = src // BANK
    w_of_edge = t_in_core // WIN
    # order edges by (core, window, bank, tile), stable
    key = ((core * NW + w_of_edge) * N_BANKS + b_of_edge) * NT + t_in_core
    nkey = N_CORES * NW * N_BANKS * NT
    counts_flat = np.bincount(key, minlength=nkey)
    t_arr = np.arange(NT)
    kk = (
        (np.arange(N_CORES)[:, None, None] * NW + (t_arr // WIN)[None, :, None])
        * N_BANKS
        + np.arange(N_BANKS)[None, None, :]
    ) * NT + t_arr[None, :, None]
    counts = counts_flat[kk]  # [N_CORES, NT, N_BANKS]
    nch = np.ceil(counts.max(axis=0) / P).astype(np.int64)  # [NT, N_BANKS]

    # global chunk offset per (t, b) group in (w, b, t) order
    goff = np.zeros((NT, N_BANKS), dtype=np.int64)
    off = 0
    for w in range(NW):
        for b in range(N_BANKS):
            for t in range(w * WIN, min((w + 1) * WIN, NT)):
                goff[t, b] = off
                off += nch[t, b]
    total_chunks = int(off)

    order = np.argsort(key, kind="stable")
    key_s = key[order]
    starts = np.zeros(nkey, dtype=np.int64)
    np.cumsum(counts_flat[:-1], out=starts[1:])
    offs = np.arange(len(order), dtype=np.int64) - starts[key_s]  # group-local

    src_s = src[order].astype(np.int64)
    dst_s = dst[order].astype(np.int64)
    core_s = core[order]
    t_s = t_in_core[order]
    b_s = b_of_edge[order]
    slot = (goff[t_s, b_s] * P + offs).astype(np.int64)  # per-core slot id
    loc = (dst_s - core_s * NPC) & 127  # local id within node tile

    shards = []
    for c in range(N_CORES):
        m = core_s == c
        qidx = np.zeros((16, total_chunks * 8), np.int16)
        dstl = np.full((P, total_chunks), -1.0, np.float32)
        s = slot[m]
        qidx[s % 16, s // 16] = (src_s[m] - b_s[m] * BANK).astype(np.int16)
        dstl[s % P, s // P] = loc[m]
        shards.append(
            {
                "qidx": np.tile(qidx, (8, 1)),
                "dstl": dstl.astype(BF16),
            }
        )
    return nch, shards


def build_program(nch):
    """Build the SPMD Bass program. nch: [NT, N_BANKS] chunk counts."""
    bass = __import__("concourse.bass", fromlist=["bass"])
    nc = bacc.Bacc(None, target_bir_lowering=False, num_swdge_queues=4)
    BT = mybir.dt.bfloat16
    entities = nc.dram_tensor(
        "entities", [N_FULL, D], BT, kind="ExternalInput"
    )
    ntT = nc.dram_tensor("ntT", [P, NLOC], BT, kind="ExternalInput")
    # chunk layout bookkeeping (all static python)
    goff = {}
    off = 0
    win_base = []
    win_runs = []  # per window: list of (b, base_chunk, run_chunks)
    for w in range(NW):
        win_base.append(off)
        runs = []
        for b in range(N_BANKS):
            rb = off
            for t in range(w * WIN, min((w + 1) * WIN, NT)):
                goff[(t, b)] = off
                off += int(nch[t, b])
            if off > rb:
                runs.append((b, rb, off - rb))
        win_runs.append(runs)
    total_chunks = off
    win_base.append(off)

    qidx = nc.dram_tensor(
        "qidx", [P, total_chunks * 8], mybir.dt.int16, kind="ExternalInput"
    )
    dstl = nc.dram_tensor("dstl", [P, total_chunks], BT, kind="ExternalInput")
    out = nc.dram_tensor("out", [NLOC, D], mybir.dt.float32, kind="ExternalOutput")

    with TileContext(nc) as tc:
        with (
            tc.tile_pool(name="const_pool", bufs=1) as cpool,
            tc.tile_pool(name="idx_pool", bufs=1) as ipool,
            tc.tile_pool(name="gather_pool", bufs=2) as gpool,
            tc.tile_pool(name="ind_pool", bufs=2) as indpool,
            tc.tile_pool(name="qvt_pool", bufs=3) as qpool,
            tc.tile_pool(name="exp_pool", bufs=3) as epool,
            tc.tile_pool(name="m_pool", bufs=3) as mpool,
            tc.tile_pool(name="work_pool", bufs=4) as wpool,
            tc.tile_pool(name="out_pool", bufs=3) as opool,
            tc.tile_pool(name="pt_pool", bufs=2, space="PSUM") as ptpool,
            tc.tile_pool(name="pa_pool", bufs=2, space="PSUM") as papool,
            tc.tile_pool(name="pw_pool", bufs=2, space="PSUM") as pwpool,
            tc.tile_pool(name="ps_pool", bufs=2, space="PSUM") as pspool,
        ):
            identity_f = cpool.tile([P, P], mybir.dt.float32)
            make_identity(nc, identity_f[:])
            identity = cpool.tile([P, P], BT)
            nc.vector.tensor_copy(identity[:], identity_f[:])
            iota_i = cpool.tile([P, P], mybir.dt.int32)
            nc.gpsimd.iota(iota_i[:], pattern=[[1, P]], base=0, channel_multiplier=0)
            iota_f = cpool.tile([P, P], BT)
            nc.vector.tensor_copy(iota_f[:], iota_i[:])
            ones = cpool.tile([P, 1], BT)
            nc.vector.memset(ones[:], 1.0)

            ntT_sb = ipool.tile([P, NLOC], BT)
            nc.sync.dma_start(out=ntT_sb[:], in_=ntT[:])
            dstl_sb = ipool.tile([P, total_chunks], BT)
            nc.sync.dma_start(out=dstl_sb[:], in_=dstl[:])
            qidx_sb = ipool.tile([P, total_chunks * 8], mybir.dt.int16)
            nc.sync.dma_start(out=qidx_sb[:], in_=qidx[:])

            for w in range(NW):
                w0 = win_base[w]
                wch = win_base[w + 1] - w0
                tiles = list(range(w * WIN, min((w + 1) * WIN, NT)))

                qv = gpool.tile([P, wch, D], BT, tag="qv", name="qv")
                for b, cb, rc in win_runs[w]:
                    ni = rc * P
                    nc.gpsimd.dma_gather(
                        qv[:, cb - w0 : cb - w0 + rc, :],
                        entities[b * BANK : min((b + 1) * BANK, N_FULL), :],
                        qidx_sb[:, cb * 8 : (cb + rc) * 8],
                        ni,
                        ni,
                        D,
                        single_packet=False,
                        queue_num=b,
                    )

                # indicator for the whole window: ind[p, c, n] = (dstl[p,c]==n)
                ind = indpool.tile([P, wch, P], BT, tag="ind", name="ind")
                nc.vector.tensor_tensor(
                    out=ind[:],
                    in0=dstl_sb[:, w0 : w0 + wch, None].to_broadcast([P, wch, P]),
                    in1=iota_f[:, None, :].to_broadcast([P, wch, P]),
                    op=mybir.AluOpType.is_equal,
                )

                # per node tile: batches of <=4 chunks within each bank run
                for t in tiles:
                    batches = []
                    for b in range(N_BANKS):
                        g0 = goff[(t, b)] - w0  # window-local chunk base
                        gn = int(nch[t, b])
                        for s in range(g0, g0 + gn, 4):
                            batches.append((s, min(4, g0 + gn - s)))
                    tnch = sum(n for _, n in batches)
                    if tnch == 0:
                        zt = opool.tile([P, D], mybir.dt.float32, tag="ot",
                                        name="ot")
                        nc.vector.memset(zt[:], 0.0)
                        nc.sync.dma_start(
                            out=out[t * P : (t + 1) * P, :], in_=zt[:]
                        )
                        continue
                    accw = pwpool.tile([P, D], mybir.dt.float32,
                                       tag="accw", name="accw")
                    accs = pspool.tile([P, 1], mybir.dt.float32,
                                       tag="accs", name="accs")
                    done = 0
                    for s, rn in batches:
                        # transpose rn chunks into one PSUM bank (bf16)
                        tp = ptpool.tile([P, 4, P], BT, tag="tp", name="tp")
                        for j in range(rn):
                            nc.tensor.transpose(
                                tp[:, j, :], qv[:, s + j, :], identity[:]
                            )
                        qvT = qpool.tile([P, 4, P], BT, tag="qvT", name="qvT")
                        nc.vector.tensor_copy(qvT[:, :rn, :], tp[:, :rn, :])
                        # scores for rn chunks into one PSUM bank (fp32)
                        ap = papool.tile([P, 4, P], mybir.dt.float32,
                                         tag="ap", name="ap")
                        for j in range(rn):
                            nc.tensor.matmul(
                                ap[:, j, :],
                                lhsT=qvT[:, j, :],
                                rhs=ntT_sb[:, t * P : (t + 1) * P],
                                start=True,
                                stop=True,
                            )
                        expa = epool.tile([P, 4, P], BT, tag="expa", name="expa")
                        nc.scalar.activation(
                            expa[:, :rn, :],
                            ap[:, :rn, :],
                            mybir.ActivationFunctionType.Exp,
                            scale=SCALE,
                        )
                        msel = mpool.tile([P, 4, P], BT, tag="msel", name="msel")
                        nc.vector.tensor_tensor(
                            out=msel[:, :rn, :],
                            in0=expa[:, :rn, :],
                            in1=ind[:, s : s + rn, :],
                            op=mybir.AluOpType.mult,
                        )
                        for j in range(rn):
                            first = done == 0
                            last = done == tnch - 1
                            nc.tensor.matmul(
                                accw[:],
                                lhsT=msel[:, j, :],
                                rhs=qv[:, s + j, :],
                                start=first,
                                stop=last,
                            )
                            nc.tensor.matmul(
                                accs[:],
                                lhsT=msel[:, j, :],
                                rhs=ones[:],
                                start=first,
                                stop=last,
                            )
                            done += 1
                    denom = wpool.tile([P, 1], mybir.dt.float32, tag="den",
                                       name="den")
                    nc.vector.tensor_scalar_add(denom[:], accs[:], EPS)
                    recip = wpool.tile([P, 1], mybir.dt.float32, tag="rec",
                                       name="rec")
                    nc.vector.reciprocal(recip[:], denom[:])
                    ot = opool.tile([P, D], mybir.dt.float32, tag="ot", name="ot")
                    nc.scalar.activation(
                        ot[:],
                        accw[:],
                        mybir.ActivationFunctionType.Copy,
                        scale=recip[:],
                    )
                    nc.sync.dma_start(out=out[t * P : (t + 1) * P, :], in_=ot[:])
    nc.compile()
    return nc


def kernel(entities, relations, edge_index, _trace=False):
    entities = np.ascontiguousarray(entities, dtype=np.float32)
    src = np.asarray(edge_index[0], dtype=np.int64)
    dst = np.asarray(edge_index[2], dtype=np.int64)
    assert entities.shape == (N_FULL, D)

    nch, shards = _prep_shards(src, dst)
    nc = build_program(nch)

    ent_bf16 = entities.astype(BF16)
    in_maps = []
    for c in range(N_CORES):
        ntT_c = np.ascontiguousarray(
            np.pad(
                entities[c * NPC : (c + 1) * NPC], ((0, NLOC - NPC), (0, 0))
            ).T
        ).astype(BF16)
        in_maps.append(
            {
                "entities": ent_bf16,
                "ntT": ntT_c,
                "qidx": shards[c]["qidx"],
                "dstl": shards[c]["dstl"],
            }
        )
    res = run_bass_kernel_spmd(
        nc, in_maps, core_ids=list(range(N_CORES)), trace=_trace
    )
    out = np.concatenate([r["out"][:NPC] for r in res.results], axis=0)
    if _trace:
        kernel.last_results = res
    return out
